# revision 2
# baseline (speedup 1.0000x reference)
"""Causal multi-head self-attention with RoPE on 8 Trainium2 NeuronCores.

Sharding: batch (4) x head-half (2) -> 8 cores, no device collectives.
Core (b, j) computes heads 8j..8j+7 of batch b for ALL 2048 query rows.
K/V/Q projections are head-sliced (512 of 1024 dims) so nothing is
duplicated across cores.  The output projection is row-sharded over Wo;
each core emits a partial y[2048, 1024] and the host sums the two halves
of each batch at gather time (the all-reduce of the sharding hint, done
during unshard).

Single fused pipeline, bf16 matmuls (fp32 PSUM):
  per 512-seq chunk a: K^T proj+RoPE (4 head-pair groups), V proj,
  Q^T proj+RoPE, then attention pair a = q-tiles (2a, 2a+1).  All of
  chunk a+1's projection work is emitted as PE filler inside attention
  pair a, together with output-projection groups of finished pairs, so
  the tensor engine never idles while the scalar engine works through
  the exp stream.
  Attention uses a transposed AV form: scores S = Krot_blk^T.T @ Qrot
  land as [128 keys, q]; exp for both heads of a pair is ONE activation
  over a 2-bank PSUM tile; then A = exp^T @ [V|1] is computed with exp
  as the STATIONARY operand so the output [128 q, 65] uses all 128
  output partitions (half the PE cycles of the feature-major form) and
  the softmax denominator lands on the free dim, making normalization a
  single free-dim-broadcast multiply on DVE.  Normalized [q, feat] tiles
  are flipped back to feature-major via DMA-engine xbar transposes
  (zero compute-engine cost) for the output projection.
  Causal masking is multiplicative on the exp weights.  Each q tile's
  last two key blocks are the masked ones: the triA block keeps a
  128x128 lower triangle on its first q-half, the triB block's first
  q-half is entirely dead (those score columns are never computed) and
  its second q-half keeps the same 128x128 triangle — so one triangle
  mask serves every case and triB scores/exps are 128 columns narrower.
"""

import os
import sys
import math

if "/opt/trn_rl_repo" not in sys.path:
    sys.path.append("/opt/trn_rl_repo")

import numpy as np
import ml_dtypes

import concourse.bass as bass
import concourse.tile as tile
from concourse import bacc, mybir
from concourse.bass import broadcast_tensor_aps
from concourse.bass_utils import run_bass_kernel_spmd

B = 4
S = 2048
D = 1024
H = 16            # total heads
HC = 8            # heads per core
NEP = HC // 2     # head-pair groups per core (128-partition groups)
DK = 64
VW = DK + 1       # V columns per head incl. trailing ones column
KB = 128          # key block
QT = 256          # q tile
THETA = 10000.0

F32 = mybir.dt.float32
BF16 = mybir.dt.bfloat16
EXPF = mybir.ActivationFunctionType.Exp

_cache = {}


def _build_program():
    if "nc" in _cache:
        return _cache["nc"]

    nc = bacc.Bacc("TRN2")

    xt_d = nc.dram_tensor("xt", [D, S], BF16, kind="ExternalInput")
    wkt_d = nc.dram_tensor("wkt", [D, D // 2], BF16, kind="ExternalInput")
    wvt_d = nc.dram_tensor("wvt", [D, D // 2], BF16, kind="ExternalInput")
    wqt_d = nc.dram_tensor("wqt", [D, D // 2], BF16, kind="ExternalInput")
    wot_d = nc.dram_tensor("wot", [D // 2, D], BF16, kind="ExternalInput")
    cosk_d = nc.dram_tensor("cosk", [128, S], BF16, kind="ExternalInput")
    sink_d = nc.dram_tensor("sink", [128, S], BF16, kind="ExternalInput")
    mask_d = nc.dram_tensor("mask", [128, 128], BF16, kind="ExternalInput")
    permt_d = nc.dram_tensor("permt", [128, 128], BF16, kind="ExternalInput")
    y_d = nc.dram_tensor("y", [S, D], F32, kind="ExternalOutput")

    xt_t = xt_d.rearrange("(n p) s -> p n s", p=128)
    wkt_t = wkt_d.rearrange("(n p) e -> p n e", p=128)
    wqt_t = wqt_d.rearrange("(n p) e -> p n e", p=128)

    with tile.TileContext(nc) as tc:
        with (
            tc.tile_pool(name="wgt", bufs=1) as wgt,
            tc.tile_pool(name="kvq", bufs=1) as kvq,
            tc.tile_pool(name="tab", bufs=1) as tab,
            tc.tile_pool(name="xsp", bufs=2) as xsp,
            tc.tile_pool(name="wrk", bufs=2) as wrk,
            tc.tile_pool(name="epl", bufs=3) as epl,
            tc.tile_pool(name="anp", bufs=2) as anp,
            tc.tile_pool(name="nrm", bufs=2) as nrm,
            tc.tile_pool(name="ocp", bufs=2) as ocp,
            tc.tile_pool(name="ps2", bufs=1, space="PSUM") as ps2,
            tc.tile_pool(name="psa", bufs=1, space="PSUM") as psa,
        ):
            # ---- persistent SBUF tiles ----
            permt = wgt.tile([128, 128], BF16, tag="permt", bufs=1)
            wk = [wgt.tile([128, 8, 128], BF16, tag=f"wk{e}", bufs=1,
                           name=f"wk{e}") for e in range(NEP)]
            wq = [wgt.tile([128, 8, 128], BF16, tag=f"wq{e}", bufs=1,
                           name=f"wq{e}") for e in range(NEP)]
            wv = [wgt.tile([128, 512], BF16, tag=f"wv{d}", bufs=1,
                           name=f"wv{d}") for d in range(8)]
            wo = [wgt.tile([128, D], BF16, tag=f"wo{e}", bufs=1,
                           name=f"wo{e}") for e in range(NEP)]
            krot = [kvq.tile([128, S], BF16, tag=f"krot{e}", bufs=1,
                             name=f"krot{e}") for e in range(NEP)]
            qrot = [kvq.tile([128, S], BF16, tag=f"qrot{e}", bufs=1,
                             name=f"qrot{e}") for e in range(NEP)]
            vt = [kvq.tile([128, HC, VW], BF16, tag=f"vt{k}", bufs=1,
                           name=f"vt{k}") for k in range(S // KB)]
            aTT = [kvq.tile([128, 16, 128], BF16, tag=f"aTT{e}", bufs=1,
                            name=f"aTT{e}") for e in range(NEP)]
            cosk = tab.tile([128, S], BF16, tag="cosk", bufs=1)
            sink = tab.tile([128, S], BF16, tag="sink", bufs=1)
            tri = tab.tile([128, 128], BF16, tag="tri", bufs=1)

            # ---- input DMAs, in priority order ----
            # First chunk's K-projection operands feed from both queues so
            # the PE can start ~1us in.
            xs_first = xsp.tile([128, 8, 512], BF16, tag="xs", bufs=2,
                                name="xs_first")
            for dd in range(4):
                nc.sync.dma_start(wk[0][:, 2 * dd:2 * dd + 2, :],
                                  wkt_t[:, 2 * dd:2 * dd + 2, 0:128])
                nc.sync.dma_start(xs_first[:, 2 * dd:2 * dd + 2, :],
                                  xt_t[:, 2 * dd:2 * dd + 2, 0:512])
            for e in range(1, NEP):
                nc.sync.dma_start(wk[e][:], wkt_t[:, :, e * 128:(e + 1) * 128])
            nc.sync.dma_start(cosk[:], cosk_d[:])
            nc.sync.dma_start(sink[:], sink_d[:])
            nc.sync.dma_start(permt[:], permt_d[:])
            for d in range(8):
                nc.sync.dma_start(wv[d][:], wvt_d[d * 128:(d + 1) * 128, :])
            for e in range(NEP):
                nc.sync.dma_start(wq[e][:], wqt_t[:, :, e * 128:(e + 1) * 128])
            nc.sync.dma_start(tri[:], mask_d[:])
            for e in range(NEP):
                nc.sync.dma_start(wo[e][:], wot_d[e * 128:(e + 1) * 128, :])

            # ones columns of vt (softmax denominator rows of the AV matmul)
            for k in range(S // KB):
                nc.vector.memset(vt[k][:, :, DK], 1.0)

            # ---- helpers ----
            def k_rope(kraw, e, csl):
                pp = ps2.tile([128, 512], F32, tag="proj", bufs=2, name="ppk")
                nc.tensor.matmul(pp[:], permt[:], kraw[:], start=True,
                                 stop=True)
                t_c = wrk.tile([128, 512], BF16, tag="t_c", bufs=2, name="t_c")
                nc.vector.tensor_mul(t_c[:], kraw[:], cosk[:, csl])
                t_s = wrk.tile([128, 512], BF16, tag="t_s", bufs=2, name="t_s")
                nc.vector.tensor_mul(t_s[:], pp[:], sink[:, csl])
                nc.vector.tensor_add(krot[e][:, csl], t_c[:], t_s[:])

            def q_rope(qraw, e, csl):
                pp = ps2.tile([128, 512], F32, tag="proj", bufs=2, name="ppq")
                nc.tensor.matmul(pp[:], permt[:], qraw[:], start=True,
                                 stop=True)
                t_c = wrk.tile([128, 512], BF16, tag="qt_c", bufs=2,
                               name="qt_c")
                nc.vector.tensor_mul(t_c[:], qraw[:], cosk[:, csl])
                t_s = wrk.tile([128, 512], BF16, tag="qt_s", bufs=2,
                               name="qt_s")
                nc.vector.tensor_mul(t_s[:], pp[:], sink[:, csl])
                nc.vector.tensor_add(qrot[e][:, csl], t_c[:], t_s[:])

            def vproj_kb(xs, kb, cpy=None):
                # V^T projection for key block kb: [128 seq, 512 feat]
                off = (kb % 4) * KB
                cp = cpy or nc.vector.tensor_copy

                def emit():
                    pv = ps2.tile([128, 512], F32, tag="proj", bufs=2,
                                  name="pv")
                    for d in range(8):
                        nc.tensor.matmul(
                            pv[:], xs[:, d, off:off + KB], wv[d][:],
                            start=(d == 0), stop=(d == 7),
                        )
                    cp(
                        vt[kb][:, :, 0:DK],
                        pv[:].rearrange("p (h w) -> p h w", w=DK),
                    )
                return emit

            def oproj_group(qs, et, tail=False):
                def emit():
                    po = ps2.tile([128, 512], F32, tag="proj", bufs=2,
                                  name="po")
                    for e2 in range(NEP):
                        nc.tensor.matmul(
                            po[:], aTT[e2][:, qs, :],
                            wo[e2][:, et * 512:(et + 1) * 512],
                            start=(e2 == 0), stop=(e2 == NEP - 1),
                        )
                    ot = ocp.tile([128, 512], F32, tag="ot", bufs=2,
                                  name="ot")
                    nc.vector.tensor_copy(ot[:], po[:])
                    nc.sync.dma_start(
                        y_d[qs * 128:(qs + 1) * 128,
                            et * 512:(et + 1) * 512],
                        ot[:],
                    )
                return emit

            def attention_pair(a, fillers):
                # q tiles (2a, 2a+1): Ca = 4a+2 and Cb = 4a+4 key blocks.
                # kb classes: plain joint (< Ca-2), joint triA (Ca-2), joint
                # triB (Ca-1, 384 wide), solo triA (Cb-2, 256), solo triB
                # (Cb-1, 128).
                Ca, Cb = 4 * a + 2, 4 * a + 4
                q0 = a * 512
                for e in range(NEP):
                    acc_a = psa.tile([128, 2, 2, VW], F32, tag="acc", bufs=2,
                                     name="acc_a")
                    acc_b = psa.tile([128, 2, 2, VW], F32, tag="acc", bufs=2,
                                     name="acc_b")
                    a_norm = anp.tile([128, 4, 2, DK], BF16, tag="anorm",
                                      bufs=2, name="a_norm")
                    started = {id(acc_a): False, id(acc_b): False}
                    pend = []

                    def scores(kb, ncol, coff):
                        """[128, 2, ncol] exp'd scores for q cols
                        [q0+coff, q0+coff+ncol)."""
                        psc = ps2.tile([128, 2, 512], F32, tag="sc",
                                       bufs=2, name="psc")
                        for h in range(2):
                            pb_ = h * DK
                            nc.tensor.matmul(
                                psc[:, h, 0:ncol],
                                krot[e][pb_:pb_ + DK,
                                        kb * KB:(kb + 1) * KB],
                                qrot[e][pb_:pb_ + DK,
                                        q0 + coff:q0 + coff + ncol],
                                start=True, stop=True,
                                tile_position=(pb_, 0),
                            )
                        ex = epl.tile([128, 2, 512], BF16, tag="ex",
                                      bufs=6, name="ex")
                        nc.scalar.activation(ex[:, :, 0:ncol],
                                             psc[:, :, 0:ncol], EXPF,
                                             scale=1.0 / math.sqrt(DK))
                        return ex

                    def masked(ex, coff):
                        # multiply ex[:, :, coff:coff+128] by the triangle
                        em = epl.tile([128, 2, KB], BF16, tag="em",
                                      bufs=5, name="em")
                        i0, i1 = broadcast_tensor_aps(
                            ex[:, :, coff:coff + KB], tri[:, None, :])
                        nc.vector.tensor_mul(em[:], i0, i1)
                        return em

                    def flush_av(kb, items):
                        # items: (acc, g, h, weight_ap); the last write of
                        # acc_a is at kb Ca-1, of acc_b at Cb-1.
                        final = {}
                        for n, (acc, g, h, wap) in enumerate(items):
                            final[id(acc)] = n
                        for n, (acc, g, h, wap) in enumerate(items):
                            st_ = not started[id(acc)]
                            started[id(acc)] = True
                            C = Ca if acc is acc_a else Cb
                            nc.tensor.matmul(
                                acc[:, g, h, :], wap, vt[kb][:, 2 * e + h, :],
                                start=st_,
                                stop=(kb == C - 1 and n == final[id(acc)]),
                            )

                    for kb in range(Cb):
                        items = []
                        if kb < Ca - 2:            # plain joint
                            ex = scores(kb, 512, 0)
                            for h in range(2):
                                for g in range(2):
                                    items.append(
                                        (acc_a, g, h,
                                         ex[:, h, g * KB:(g + 1) * KB]))
                                for g in range(2):
                                    items.append(
                                        (acc_b, g, h,
                                         ex[:, h, QT + g * KB:
                                            QT + (g + 1) * KB]))
                        elif kb == Ca - 2:         # joint triA
                            ex = scores(kb, 512, 0)
                            em = masked(ex, 0)
                            for h in range(2):
                                items.append((acc_a, 0, h, em[:, h, :]))
                                items.append(
                                    (acc_a, 1, h, ex[:, h, KB:QT]))
                                for g in range(2):
                                    items.append(
                                        (acc_b, g, h,
                                         ex[:, h, QT + g * KB:
                                            QT + (g + 1) * KB]))
                        elif kb == Ca - 1:         # joint triB (384 wide)
                            ex = scores(kb, 384, KB)
                            em = masked(ex, 0)
                            for h in range(2):
                                items.append((acc_a, 1, h, em[:, h, :]))
                                for g in range(2):
                                    items.append(
                                        (acc_b, g, h,
                                         ex[:, h, KB + g * KB:
                                            KB + (g + 1) * KB]))
                        elif kb == Cb - 2:         # solo triA
                            ex = scores(kb, 256, 256)
                            em = masked(ex, 0)
                            for h in range(2):
                                items.append((acc_b, 0, h, em[:, h, :]))
                                items.append(
                                    (acc_b, 1, h, ex[:, h, KB:QT]))
                        else:                      # solo triB
                            ex = scores(kb, 128, 384)
                            em = masked(ex, 0)
                            for h in range(2):
                                items.append((acc_b, 1, h, em[:, h, :]))
                        pend.append((kb, items))
                        if len(pend) > 3:
                            flush_av(*pend.pop(0))
                        if fillers:
                            fillers.pop(0)()
                    for p_ in pend:
                        flush_av(*p_)

                    def normalize(acc, tl):
                        rb = nrm.tile([128, 2, 2, 1], F32, tag="rb", bufs=2,
                                      name="rb")
                        nc.vector.reciprocal(rb[:], acc[:, :, :, DK:VW])
                        i0, i1 = broadcast_tensor_aps(acc[:, :, :, 0:DK],
                                                      rb[:])
                        nc.vector.tensor_mul(
                            a_norm[:, 2 * tl:2 * tl + 2, :, :], i0, i1)

                    normalize(acc_a, 0)
                    normalize(acc_b, 1)
                    # feature-major flip via DMA xbar transpose
                    nc.sync.dma_start_transpose(
                        aTT[e][:, 4 * a:4 * a + 4, :],
                        a_norm[:].rearrange("p g h d -> p (g h d)"),
                    )
                    if fillers:
                        fillers.pop(0)()
                for f in fillers:
                    f()
                del fillers[:]

            # ---- fused projection + attention pipeline ----
            def chunk_work(st, xs):
                """Closures for chunk st's K/V/Q projections + RoPE, to be
                interleaved as PE filler inside the previous attention
                pair (or run serially for chunk 0)."""
                csl = slice(st * 512, (st + 1) * 512)
                pend, pendq = [], []
                # chunk 3's PSUM->SBUF copies run inside the ACT-saturated
                # deep pairs: use DVE there, ACT (idle early) otherwise
                cpy = nc.vector.tensor_copy if st == 3 else nc.scalar.copy

                def kproj(e):
                    def go():
                        pk = ps2.tile([128, 512], F32, tag="proj", bufs=2,
                                      name="pk")
                        for d in range(8):
                            nc.tensor.matmul(
                                pk[:], wk[e][:, d, :], xs[:, d, :],
                                start=(d == 0), stop=(d == 7),
                            )
                        kraw = wrk.tile([128, 512], BF16, tag="kraw", bufs=2,
                                        name="kraw")
                        cpy(kraw[:], pk[:])
                        pend.append((kraw, e, csl))
                        if len(pend) > 1:
                            k_rope(*pend.pop(0))
                    return go

                def qproj(e):
                    def go():
                        if pend:
                            k_rope(*pend.pop(0))
                        pq = ps2.tile([128, 512], F32, tag="proj", bufs=2,
                                      name="pq")
                        for d in range(8):
                            nc.tensor.matmul(
                                pq[:], wq[e][:, d, :], xs[:, d, :],
                                start=(d == 0), stop=(d == 7),
                            )
                        qraw = wrk.tile([128, 512], BF16, tag="qraw", bufs=2,
                                        name="qraw")
                        cpy(qraw[:], pq[:])
                        pendq.append((qraw, e, csl))
                        if len(pendq) > 1:
                            q_rope(*pendq.pop(0))
                    return go

                def flush():
                    while pend:
                        k_rope(*pend.pop(0))
                    while pendq:
                        q_rope(*pendq.pop(0))

                if st == 0:
                    # serial warmup: keep K first so V/Q weight DMAs land
                    # before their consumers
                    work = [kproj(0), kproj(1), kproj(2), kproj(3),
                            vproj_kb(xs, 0, cpy), vproj_kb(xs, 1, cpy),
                            vproj_kb(xs, 2, cpy), vproj_kb(xs, 3, cpy),
                            qproj(0), qproj(1), qproj(2), qproj(3),
                            flush]
                else:
                    work = [kproj(0), kproj(1),
                            vproj_kb(xs, 4 * st + 0, cpy),
                            kproj(2),
                            vproj_kb(xs, 4 * st + 1, cpy),
                            kproj(3),
                            vproj_kb(xs, 4 * st + 2, cpy),
                            vproj_kb(xs, 4 * st + 3, cpy),
                            qproj(0), qproj(1), qproj(2), qproj(3),
                            flush]
                return work

            # chunk 0 runs serially (nothing to overlap with yet)
            xs_tiles = [xs_first, None, None, None]
            for w in chunk_work(0, xs_first):
                w()

            # oproj groups of pair p are deferred: pair 2 absorbs pair 0's
            # and pair 3 absorbs pairs 1's and 2's, keeping the tensor
            # engine fed through the deepest (most ACT-bound) pair.
            for a in range(4):
                fillers = []
                if a < 3:
                    xs_n = xsp.tile([128, 8, 512], BF16, tag="xs", bufs=2,
                                    name="xs_n")
                    nc.gpsimd.dma_start(
                        xs_n[:], xt_t[:, :, (a + 1) * 512:(a + 2) * 512])
                    xs_tiles[a + 1] = xs_n
                    fillers += chunk_work(a + 1, xs_n)
                oout = []
                if a == 3:
                    oout = [oproj_group(qs, et)
                            for qs in range(2, 12) for et in range(2)]
                merged = []
                while fillers or oout:
                    if fillers:
                        merged.append(fillers.pop(0))
                    if oout:
                        merged.append(oout.pop(0))
                attention_pair(a, merged)

            # ---- remaining output projection ----
            # held-back early groups first: they have no dependency on pair
            # 3's final transpose, covering its normalize+transpose latency
            for qs in range(0, 2):
                for et in range(2):
                    oproj_group(qs, et)()
            for qs in range(12, 16):
                for et in range(2):
                    oproj_group(qs, et, tail=True)()

    nc.compile()
    nc.finalize()
    _cache["nc"] = nc
    return nc


def _rope_tables(pos):
    """cos/sin tables in [128, n] head-pair layout."""
    k = np.arange(DK // 2, dtype=np.float32)
    inv_freq = (THETA ** (-2.0 * k / DK)).astype(np.float32)
    ang = inv_freq[:, None] * pos.astype(np.float32)[None, :]  # [32, n]
    cos64 = np.repeat(np.cos(ang), 2, axis=0)
    sin64 = np.repeat(np.sin(ang), 2, axis=0)
    cos = np.concatenate([cos64, cos64], axis=0)
    sin = np.concatenate([sin64, sin64], axis=0)
    return (np.ascontiguousarray(cos).astype(ml_dtypes.bfloat16),
            np.ascontiguousarray(sin).astype(ml_dtypes.bfloat16))


def _host_inputs(in_features, token_positions, Wq, Wk, Wv, Wo):
    X = np.asarray(in_features, dtype=np.float32)
    pos = np.asarray(token_positions)
    bf = ml_dtypes.bfloat16
    wqt = np.ascontiguousarray(np.asarray(Wq, np.float32).T).astype(bf)
    wkt = np.ascontiguousarray(np.asarray(Wk, np.float32).T).astype(bf)
    wvt = np.ascontiguousarray(np.asarray(Wv, np.float32).T).astype(bf)
    wot = np.ascontiguousarray(np.asarray(Wo, np.float32).T).astype(bf)
    cosk, sink = _rope_tables(pos)

    # 128x128 lower triangle (key partition p live for q col f when f >= p)
    p = np.arange(KB)[:, None]
    f = np.arange(KB)[None, :]
    tri = np.ascontiguousarray((f >= p).astype(np.float32)).astype(bf)

    permt = np.zeros((128, 128), np.float32)
    for i in range(64):
        permt[2 * i + 1, 2 * i] = -1.0
        permt[2 * i, 2 * i + 1] = 1.0
    permt = permt.astype(bf)

    in_maps = []
    for core in range(8):
        b, j = core // 2, core % 2
        cols = slice(j * 512, (j + 1) * 512)
        in_maps.append({
            "xt": np.ascontiguousarray(X[b].T).astype(bf),
            "wkt": np.ascontiguousarray(wkt[:, cols]),
            "wvt": np.ascontiguousarray(wvt[:, cols]),
            "wqt": np.ascontiguousarray(wqt[:, cols]),
            "wot": np.ascontiguousarray(wot[cols, :]),
            "cosk": cosk, "sink": sink,
            "mask": tri, "permt": permt,
        })
    return in_maps


def kernel(in_features, token_positions, Wq, Wk, Wv, Wo):
    nc = _build_program()
    in_maps = _host_inputs(in_features, token_positions, Wq, Wk, Wv, Wo)

    trace = bool(int(os.environ.get("KERNEL_TRACE", "0")))
    res = run_bass_kernel_spmd(nc, in_maps, core_ids=list(range(8)),
                               trace=trace)
    kernel.last_result = res

    out = np.empty((B, S, D), np.float32)
    for b in range(B):
        out[b] = res.results[2 * b]["y"] + res.results[2 * b + 1]["y"]
    return out


# revision 3
# speedup vs baseline: 1.0328x; 1.0328x over previous
"""Causal multi-head self-attention with RoPE on 8 Trainium2 NeuronCores.

Sharding: batch (4) x head-half (2) -> 8 cores, no device collectives.
Core (b, j) computes heads 8j..8j+7 of batch b for ALL 2048 query rows.
K/V/Q projections are head-sliced (512 of 1024 dims) so nothing is
duplicated across cores.  The output projection is row-sharded over Wo;
each core emits a partial y[2048, 1024] and the host sums the two halves
of each batch at gather time (the all-reduce of the sharding hint, done
during unshard).

Single fused pipeline, bf16 matmuls (fp32 PSUM):
  per 512-seq chunk a: K^T proj+RoPE (4 head-pair groups), V proj,
  Q^T proj+RoPE, then attention pair a = q-tiles (2a, 2a+1).  All of
  chunk a+1's projection work is emitted as PE filler inside attention
  pair a, together with output-projection groups of finished pairs, so
  the tensor engine never idles while the scalar engine works through
  the exp stream.
  Attention uses a transposed AV form: scores S = Krot_blk^T.T @ Qrot
  land as [128 keys, q]; exp for both heads of a pair is ONE activation
  over a 2-bank PSUM tile; then A = exp^T @ [V|1] is computed with exp
  as the STATIONARY operand so the output [128 q, 65] uses all 128
  output partitions (half the PE cycles of the feature-major form) and
  the softmax denominator lands on the free dim, making normalization a
  single free-dim-broadcast multiply on DVE.  Normalized [q, feat] tiles
  are flipped back to feature-major via DMA-engine xbar transposes
  (zero compute-engine cost) for the output projection.
  Causal masking is multiplicative on the exp weights.  Each q tile's
  last two key blocks are the masked ones: the triA block keeps a
  128x128 lower triangle on its first q-half, the triB block's first
  q-half is entirely dead (those score columns are never computed) and
  its second q-half keeps the same 128x128 triangle — so one triangle
  mask serves every case and triB scores/exps are 128 columns narrower.
"""

import os
import sys
import math

if "/opt/trn_rl_repo" not in sys.path:
    sys.path.append("/opt/trn_rl_repo")

import numpy as np
import ml_dtypes

import concourse.bass as bass
import concourse.tile as tile
from concourse import bacc, mybir
from concourse.bass import broadcast_tensor_aps
from concourse.bass_utils import run_bass_kernel_spmd

B = 4
S = 2048
D = 1024
H = 16            # total heads
HC = 8            # heads per core
NEP = HC // 2     # head-pair groups per core (128-partition groups)
DK = 64
VW = DK + 1       # V columns per head incl. trailing ones column
KB = 128          # key block
QT = 256          # q tile
THETA = 10000.0

F32 = mybir.dt.float32
BF16 = mybir.dt.bfloat16
EXPF = mybir.ActivationFunctionType.Exp

_cache = {}


def _build_program():
    if "nc" in _cache:
        return _cache["nc"]

    nc = bacc.Bacc("TRN2")

    xt_d = nc.dram_tensor("xt", [D, S], BF16, kind="ExternalInput")
    wkt_d = nc.dram_tensor("wkt", [D, D // 2], BF16, kind="ExternalInput")
    wvt_d = nc.dram_tensor("wvt", [D, D // 2], BF16, kind="ExternalInput")
    wqt_d = nc.dram_tensor("wqt", [D, D // 2], BF16, kind="ExternalInput")
    wot_d = nc.dram_tensor("wot", [D // 2, D], BF16, kind="ExternalInput")
    cosk_d = nc.dram_tensor("cosk", [128, S], BF16, kind="ExternalInput")
    sink_d = nc.dram_tensor("sink", [128, S], BF16, kind="ExternalInput")
    mask_d = nc.dram_tensor("mask", [128, 128], BF16, kind="ExternalInput")
    permt_d = nc.dram_tensor("permt", [128, 128], BF16, kind="ExternalInput")
    y_d = nc.dram_tensor("y", [S, D], F32, kind="ExternalOutput")

    xt_t = xt_d.rearrange("(n p) s -> p n s", p=128)
    wkt_t = wkt_d.rearrange("(n p) e -> p n e", p=128)
    wqt_t = wqt_d.rearrange("(n p) e -> p n e", p=128)

    with tile.TileContext(nc) as tc:
        with (
            tc.tile_pool(name="wgt", bufs=1) as wgt,
            tc.tile_pool(name="kvq", bufs=1) as kvq,
            tc.tile_pool(name="tab", bufs=1) as tab,
            tc.tile_pool(name="xsp", bufs=2) as xsp,
            tc.tile_pool(name="wrk", bufs=2) as wrk,
            tc.tile_pool(name="epl", bufs=3) as epl,
            tc.tile_pool(name="anp", bufs=2) as anp,
            tc.tile_pool(name="nrm", bufs=2) as nrm,
            tc.tile_pool(name="ocp", bufs=2) as ocp,
            tc.tile_pool(name="ps2", bufs=1, space="PSUM") as ps2,
            tc.tile_pool(name="psa", bufs=1, space="PSUM") as psa,
        ):
            # ---- persistent SBUF tiles ----
            permt = wgt.tile([128, 128], BF16, tag="permt", bufs=1)
            wk = [wgt.tile([128, 8, 128], BF16, tag=f"wk{e}", bufs=1,
                           name=f"wk{e}") for e in range(NEP)]
            wq = [wgt.tile([128, 8, 128], BF16, tag=f"wq{e}", bufs=1,
                           name=f"wq{e}") for e in range(NEP)]
            wv = [wgt.tile([128, 512], BF16, tag=f"wv{d}", bufs=1,
                           name=f"wv{d}") for d in range(8)]
            wo = [wgt.tile([128, D], BF16, tag=f"wo{e}", bufs=1,
                           name=f"wo{e}") for e in range(NEP)]
            krot = [kvq.tile([128, S], BF16, tag=f"krot{e}", bufs=1,
                             name=f"krot{e}") for e in range(NEP)]
            qrot = [kvq.tile([128, S], BF16, tag=f"qrot{e}", bufs=1,
                             name=f"qrot{e}") for e in range(NEP)]
            vt = [kvq.tile([128, HC, VW], BF16, tag=f"vt{k}", bufs=1,
                           name=f"vt{k}") for k in range(S // KB)]
            aTT = [[kvq.tile([128, 4, 128], BF16, tag=f"aTT{p}_{e}",
                             bufs=1, name=f"aTT{p}_{e}")
                    for e in range(NEP)] for p in range(4)]
            cosk = tab.tile([128, S], BF16, tag="cosk", bufs=1)
            sink = tab.tile([128, S], BF16, tag="sink", bufs=1)
            tri = tab.tile([128, 128], BF16, tag="tri", bufs=1)

            # ---- input DMAs, in priority order ----
            # First chunk's K-projection operands feed from both queues so
            # the PE can start ~1us in.
            xs_first = xsp.tile([128, 8, 512], BF16, tag="xs", bufs=2,
                                name="xs_first")
            for dd in range(4):
                nc.sync.dma_start(wk[0][:, 2 * dd:2 * dd + 2, :],
                                  wkt_t[:, 2 * dd:2 * dd + 2, 0:128])
                nc.sync.dma_start(xs_first[:, 2 * dd:2 * dd + 2, :],
                                  xt_t[:, 2 * dd:2 * dd + 2, 0:512])
            nc.gpsimd.dma_start(cosk[:], cosk_d[:])
            nc.gpsimd.dma_start(sink[:], sink_d[:])
            nc.gpsimd.dma_start(permt[:], permt_d[:])
            for e in range(1, NEP):
                nc.gpsimd.dma_start(wk[e][:],
                                    wkt_t[:, :, e * 128:(e + 1) * 128])
            for d in range(8):
                nc.sync.dma_start(wv[d][:], wvt_d[d * 128:(d + 1) * 128, :])
            for e in range(NEP):
                nc.sync.dma_start(wq[e][:], wqt_t[:, :, e * 128:(e + 1) * 128])
            nc.sync.dma_start(tri[:], mask_d[:])
            for e in range(NEP):
                nc.gpsimd.dma_start(wo[e][:], wot_d[e * 128:(e + 1) * 128, :])

            # ones columns of vt (softmax denominator rows of the AV matmul)
            for k in range(S // KB):
                nc.vector.memset(vt[k][:, :, DK], 1.0)

            # ---- helpers ----
            def k_rope(kraw, e, csl):
                pp = ps2.tile([128, 512], F32, tag="proj", bufs=2, name="ppk")
                nc.tensor.matmul(pp[:], permt[:], kraw[:], start=True,
                                 stop=True)
                t_c = wrk.tile([128, 512], BF16, tag="t_c", bufs=2, name="t_c")
                nc.vector.tensor_mul(t_c[:], kraw[:], cosk[:, csl])
                t_s = wrk.tile([128, 512], BF16, tag="t_s", bufs=2, name="t_s")
                nc.vector.tensor_mul(t_s[:], pp[:], sink[:, csl])
                nc.vector.tensor_add(krot[e][:, csl], t_c[:], t_s[:])

            def q_rope(qraw, e, csl):
                pp = ps2.tile([128, 512], F32, tag="proj", bufs=2, name="ppq")
                nc.tensor.matmul(pp[:], permt[:], qraw[:], start=True,
                                 stop=True)
                t_c = wrk.tile([128, 512], BF16, tag="qt_c", bufs=2,
                               name="qt_c")
                nc.vector.tensor_mul(t_c[:], qraw[:], cosk[:, csl])
                t_s = wrk.tile([128, 512], BF16, tag="qt_s", bufs=2,
                               name="qt_s")
                nc.vector.tensor_mul(t_s[:], pp[:], sink[:, csl])
                nc.vector.tensor_add(qrot[e][:, csl], t_c[:], t_s[:])

            def vproj_kb(xs, kb, cpy=None):
                # V^T projection for key block kb: [128 seq, 512 feat]
                off = (kb % 4) * KB
                cp = cpy or nc.vector.tensor_copy

                def emit():
                    pv = ps2.tile([128, 512], F32, tag="proj", bufs=2,
                                  name="pv")
                    for d in range(8):
                        nc.tensor.matmul(
                            pv[:], xs[:, d, off:off + KB], wv[d][:],
                            start=(d == 0), stop=(d == 7),
                        )
                    cp(
                        vt[kb][:, :, 0:DK],
                        pv[:].rearrange("p (h w) -> p h w", w=DK),
                    )
                return emit

            def oproj_group(qs, et, tail=False):
                def emit():
                    po = ps2.tile([128, 512], F32, tag="proj", bufs=2,
                                  name="po")
                    for e2 in range(NEP):
                        nc.tensor.matmul(
                            po[:], aTT[qs // 4][e2][:, qs % 4, :],
                            wo[e2][:, et * 512:(et + 1) * 512],
                            start=(e2 == 0), stop=(e2 == NEP - 1),
                        )
                    ot = ocp.tile([128, 512], F32, tag="ot", bufs=2,
                                  name="ot")
                    nc.vector.tensor_copy(ot[:], po[:])
                    nc.sync.dma_start(
                        y_d[qs * 128:(qs + 1) * 128,
                            et * 512:(et + 1) * 512],
                        ot[:],
                    )
                return emit

            def attention_pair(a, fillers):
                # q tiles (2a, 2a+1): Ca = 4a+2 and Cb = 4a+4 key blocks.
                # kb classes: plain joint (< Ca-2), joint triA (Ca-2), joint
                # triB (Ca-1, 384 wide), solo triA (Cb-2, 256), solo triB
                # (Cb-1, 128).
                Ca, Cb = 4 * a + 2, 4 * a + 4
                q0 = a * 512
                for e in range(NEP):
                    acc_a = psa.tile([128, 2, 2, VW], F32, tag="acc", bufs=2,
                                     name="acc_a")
                    acc_b = psa.tile([128, 2, 2, VW], F32, tag="acc", bufs=2,
                                     name="acc_b")
                    a_norm = anp.tile([128, 4, 2, DK], BF16, tag="anorm",
                                      bufs=2, name="a_norm")
                    started = {id(acc_a): False, id(acc_b): False}
                    pend = []

                    def scores(kb, ncol, coff):
                        """[128, 2, ncol] exp'd scores for q cols
                        [q0+coff, q0+coff+ncol)."""
                        psc = ps2.tile([128, 2, 512], F32, tag="sc",
                                       bufs=2, name="psc")
                        for h in range(2):
                            pb_ = h * DK
                            nc.tensor.matmul(
                                psc[:, h, 0:ncol],
                                krot[e][pb_:pb_ + DK,
                                        kb * KB:(kb + 1) * KB],
                                qrot[e][pb_:pb_ + DK,
                                        q0 + coff:q0 + coff + ncol],
                                start=True, stop=True,
                                tile_position=(pb_, 0),
                            )
                        ex = epl.tile([128, 2, 512], BF16, tag="ex",
                                      bufs=6, name="ex")
                        nc.scalar.activation(ex[:, :, 0:ncol],
                                             psc[:, :, 0:ncol], EXPF,
                                             scale=1.0 / math.sqrt(DK))
                        return ex

                    def masked(ex, coff):
                        # multiply ex[:, :, coff:coff+128] by the triangle
                        em = epl.tile([128, 2, KB], BF16, tag="em",
                                      bufs=5, name="em")
                        i0, i1 = broadcast_tensor_aps(
                            ex[:, :, coff:coff + KB], tri[:, None, :])
                        nc.vector.tensor_mul(em[:], i0, i1)
                        return em

                    def flush_av(kb, items):
                        # items: (acc, g, h, weight_ap); the last write of
                        # acc_a is at kb Ca-1, of acc_b at Cb-1.
                        final = {}
                        for n, (acc, g, h, wap) in enumerate(items):
                            final[id(acc)] = n
                        for n, (acc, g, h, wap) in enumerate(items):
                            st_ = not started[id(acc)]
                            started[id(acc)] = True
                            C = Ca if acc is acc_a else Cb
                            nc.tensor.matmul(
                                acc[:, g, h, :], wap, vt[kb][:, 2 * e + h, :],
                                start=st_,
                                stop=(kb == C - 1 and n == final[id(acc)]),
                            )

                    for kb in range(Cb):
                        items = []
                        if kb < Ca - 2:            # plain joint
                            ex = scores(kb, 512, 0)
                            for h in range(2):
                                for g in range(2):
                                    items.append(
                                        (acc_a, g, h,
                                         ex[:, h, g * KB:(g + 1) * KB]))
                                for g in range(2):
                                    items.append(
                                        (acc_b, g, h,
                                         ex[:, h, QT + g * KB:
                                            QT + (g + 1) * KB]))
                        elif kb == Ca - 2:         # joint triA
                            ex = scores(kb, 512, 0)
                            em = masked(ex, 0)
                            for h in range(2):
                                items.append((acc_a, 0, h, em[:, h, :]))
                                items.append(
                                    (acc_a, 1, h, ex[:, h, KB:QT]))
                                for g in range(2):
                                    items.append(
                                        (acc_b, g, h,
                                         ex[:, h, QT + g * KB:
                                            QT + (g + 1) * KB]))
                        elif kb == Ca - 1:         # joint triB (384 wide)
                            ex = scores(kb, 384, KB)
                            em = masked(ex, 0)
                            for h in range(2):
                                items.append((acc_a, 1, h, em[:, h, :]))
                                for g in range(2):
                                    items.append(
                                        (acc_b, g, h,
                                         ex[:, h, KB + g * KB:
                                            KB + (g + 1) * KB]))
                        elif kb == Cb - 2:         # solo triA
                            ex = scores(kb, 256, 256)
                            em = masked(ex, 0)
                            for h in range(2):
                                items.append((acc_b, 0, h, em[:, h, :]))
                                items.append(
                                    (acc_b, 1, h, ex[:, h, KB:QT]))
                        else:                      # solo triB
                            ex = scores(kb, 128, 384)
                            em = masked(ex, 0)
                            for h in range(2):
                                items.append((acc_b, 1, h, em[:, h, :]))
                        pend.append((kb, items))
                        if len(pend) > 3:
                            flush_av(*pend.pop(0))
                        if fillers:
                            fillers.pop(0)()
                    for p_ in pend:
                        flush_av(*p_)

                    def normalize(acc, tl):
                        rb = nrm.tile([128, 2, 2, 1], F32, tag="rb", bufs=2,
                                      name="rb")
                        nc.vector.reciprocal(rb[:], acc[:, :, :, DK:VW])
                        i0, i1 = broadcast_tensor_aps(acc[:, :, :, 0:DK],
                                                      rb[:])
                        nc.vector.tensor_mul(
                            a_norm[:, 2 * tl:2 * tl + 2, :, :], i0, i1)

                    normalize(acc_a, 0)
                    normalize(acc_b, 1)
                    # feature-major flip via DMA xbar transpose
                    nc.sync.dma_start_transpose(
                        aTT[a][e][:],
                        a_norm[:].rearrange("p g h d -> p (g h d)"),
                    )
                    if fillers:
                        fillers.pop(0)()
                for f in fillers:
                    f()
                del fillers[:]

            # ---- fused projection + attention pipeline ----
            def chunk_work(st, xs):
                """Closures for chunk st's K/V/Q projections + RoPE, to be
                interleaved as PE filler inside the previous attention
                pair (or run serially for chunk 0)."""
                csl = slice(st * 512, (st + 1) * 512)
                pend, pendq = [], []
                # chunk 3's PSUM->SBUF copies run inside the ACT-saturated
                # deep pairs: use DVE there, ACT (idle early) otherwise
                cpy = nc.vector.tensor_copy if st == 3 else nc.scalar.copy

                def kproj(e):
                    def go():
                        pk = ps2.tile([128, 512], F32, tag="proj", bufs=2,
                                      name="pk")
                        for d in range(8):
                            nc.tensor.matmul(
                                pk[:], wk[e][:, d, :], xs[:, d, :],
                                start=(d == 0), stop=(d == 7),
                            )
                        kraw = wrk.tile([128, 512], BF16, tag="kraw", bufs=2,
                                        name="kraw")
                        cpy(kraw[:], pk[:])
                        pend.append((kraw, e, csl))
                        if len(pend) > 1:
                            k_rope(*pend.pop(0))
                    return go

                def qproj(e):
                    def go():
                        if pend:
                            k_rope(*pend.pop(0))
                        pq = ps2.tile([128, 512], F32, tag="proj", bufs=2,
                                      name="pq")
                        for d in range(8):
                            nc.tensor.matmul(
                                pq[:], wq[e][:, d, :], xs[:, d, :],
                                start=(d == 0), stop=(d == 7),
                            )
                        qraw = wrk.tile([128, 512], BF16, tag="qraw", bufs=2,
                                        name="qraw")
                        cpy(qraw[:], pq[:])
                        pendq.append((qraw, e, csl))
                        if len(pendq) > 1:
                            q_rope(*pendq.pop(0))
                    return go

                def flush():
                    while pend:
                        k_rope(*pend.pop(0))
                    while pendq:
                        q_rope(*pendq.pop(0))

                if st == 0:
                    # serial warmup: keep K first so V/Q weight DMAs land
                    # before their consumers
                    work = [kproj(0), kproj(1), kproj(2), kproj(3),
                            vproj_kb(xs, 0, cpy), vproj_kb(xs, 1, cpy),
                            vproj_kb(xs, 2, cpy), vproj_kb(xs, 3, cpy),
                            qproj(0), qproj(1), qproj(2), qproj(3),
                            flush]
                else:
                    work = [kproj(0), kproj(1),
                            vproj_kb(xs, 4 * st + 0, cpy),
                            kproj(2),
                            vproj_kb(xs, 4 * st + 1, cpy),
                            kproj(3),
                            vproj_kb(xs, 4 * st + 2, cpy),
                            vproj_kb(xs, 4 * st + 3, cpy),
                            qproj(0), qproj(1), qproj(2), qproj(3),
                            flush]
                return work

            # chunk 0 runs serially (nothing to overlap with yet)
            xs_tiles = [xs_first, None, None, None]
            for w in chunk_work(0, xs_first):
                w()

            # oproj groups of pair p are deferred: pair 2 absorbs pair 0's
            # and pair 3 absorbs pairs 1's and 2's, keeping the tensor
            # engine fed through the deepest (most ACT-bound) pair.
            for a in range(4):
                fillers = []
                if a < 3:
                    xs_n = xsp.tile([128, 8, 512], BF16, tag="xs", bufs=2,
                                    name="xs_n")
                    nc.gpsimd.dma_start(
                        xs_n[:], xt_t[:, :, (a + 1) * 512:(a + 2) * 512])
                    xs_tiles[a + 1] = xs_n
                    fillers += chunk_work(a + 1, xs_n)
                oout = []
                if a == 3:
                    oout = [oproj_group(qs, et)
                            for qs in range(2, 12) for et in range(2)]
                merged = []
                while fillers or oout:
                    if fillers:
                        merged.append(fillers.pop(0))
                    if oout:
                        merged.append(oout.pop(0))
                attention_pair(a, merged)

            # ---- remaining output projection ----
            # held-back early groups first: they have no dependency on pair
            # 3's final transpose, covering its normalize+transpose latency
            for qs in range(0, 2):
                for et in range(2):
                    oproj_group(qs, et)()
            for qs in range(12, 16):
                for et in range(2):
                    oproj_group(qs, et, tail=True)()

    nc.compile()
    nc.finalize()
    _cache["nc"] = nc
    return nc


def _rope_tables(pos):
    """cos/sin tables in [128, n] head-pair layout."""
    k = np.arange(DK // 2, dtype=np.float32)
    inv_freq = (THETA ** (-2.0 * k / DK)).astype(np.float32)
    ang = inv_freq[:, None] * pos.astype(np.float32)[None, :]  # [32, n]
    cos64 = np.repeat(np.cos(ang), 2, axis=0)
    sin64 = np.repeat(np.sin(ang), 2, axis=0)
    cos = np.concatenate([cos64, cos64], axis=0)
    sin = np.concatenate([sin64, sin64], axis=0)
    return (np.ascontiguousarray(cos).astype(ml_dtypes.bfloat16),
            np.ascontiguousarray(sin).astype(ml_dtypes.bfloat16))


def _host_inputs(in_features, token_positions, Wq, Wk, Wv, Wo):
    X = np.asarray(in_features, dtype=np.float32)
    pos = np.asarray(token_positions)
    bf = ml_dtypes.bfloat16
    wqt = np.ascontiguousarray(np.asarray(Wq, np.float32).T).astype(bf)
    wkt = np.ascontiguousarray(np.asarray(Wk, np.float32).T).astype(bf)
    wvt = np.ascontiguousarray(np.asarray(Wv, np.float32).T).astype(bf)
    wot = np.ascontiguousarray(np.asarray(Wo, np.float32).T).astype(bf)
    cosk, sink = _rope_tables(pos)

    # 128x128 lower triangle (key partition p live for q col f when f >= p)
    p = np.arange(KB)[:, None]
    f = np.arange(KB)[None, :]
    tri = np.ascontiguousarray((f >= p).astype(np.float32)).astype(bf)

    permt = np.zeros((128, 128), np.float32)
    for i in range(64):
        permt[2 * i + 1, 2 * i] = -1.0
        permt[2 * i, 2 * i + 1] = 1.0
    permt = permt.astype(bf)

    in_maps = []
    for core in range(8):
        b, j = core // 2, core % 2
        cols = slice(j * 512, (j + 1) * 512)
        in_maps.append({
            "xt": np.ascontiguousarray(X[b].T).astype(bf),
            "wkt": np.ascontiguousarray(wkt[:, cols]),
            "wvt": np.ascontiguousarray(wvt[:, cols]),
            "wqt": np.ascontiguousarray(wqt[:, cols]),
            "wot": np.ascontiguousarray(wot[cols, :]),
            "cosk": cosk, "sink": sink,
            "mask": tri, "permt": permt,
        })
    return in_maps


def kernel(in_features, token_positions, Wq, Wk, Wv, Wo):
    nc = _build_program()
    in_maps = _host_inputs(in_features, token_positions, Wq, Wk, Wv, Wo)

    trace = bool(int(os.environ.get("KERNEL_TRACE", "0")))
    res = run_bass_kernel_spmd(nc, in_maps, core_ids=list(range(8)),
                               trace=trace)
    kernel.last_result = res

    out = np.empty((B, S, D), np.float32)
    for b in range(B):
        out[b] = res.results[2 * b]["y"] + res.results[2 * b + 1]["y"]
    return out
